# revision 4
# baseline (speedup 1.0000x reference)
"""Trainium2 Bass kernel for FFNWithScales (SwiGLU MLP with low-rank dequant scales).

Reference computation (all fp32):
    gate_eff = gate_snapped * (gate_scale_A @ gate_scale_B)       # [8192, 2048]
    up_eff   = up_snapped   * (up_scale_A   @ up_scale_B)         # [8192, 2048]
    down_eff = down_snapped * (down_scale_A @ down_scale_B)       # [2048, 8192]
    h   = silu(gate_eff @ x) * (up_eff @ x)                       # [8192, 512]
    out = down_eff @ h                                            # [2048, 512]

Sharding (8 cores, tensor-parallel on d_ff): core c owns d_ff rows
[c*1024, (c+1)*1024) of gate/up (and the matching columns of down).
Each core computes a full-[2048, 512] partial of the down projection;
partials are summed on the host (the all-reduce step).

Device layout notes:
  - PE matmul computes psum[M,N] = lhsT[K,M].T @ rhs[K,N] with K on
    partitions, so every weight is fed with its contraction dim on
    partitions. The host pre-transposes the snapped weights (one numpy
    transpose each) because fp32 has no DMA-transpose path on TRN2.
  - The low-rank scale (A@B)^T tiles are produced on the PE (K=32
    matmuls) and multiplied into the snapped tiles on the DVE right
    before they are consumed as matmul weights. That DVE multiply
    rounds its output to float32r, which the BIR verifier requires of
    every fp32r-matmul operand; fp32r streams at 4x the rate of fp32.
"""

import numpy as np

import concourse.bass as bass
from concourse import bacc
import concourse.mybir as mybir
from concourse.tile import TileContext
from concourse.bass_utils import run_bass_kernel_spmd

P = 128
D = 2048        # d_model
FF = 8192       # d_ff (global)
S = 512         # sequence
R = 32          # rank
NCORES = 8
F = FF // NCORES          # 1024 local d_ff rows
KD = D // P               # 16 d_model chunks
KF = F // P               # 8 local d_ff chunks
FG = 512                  # free-dim group (psum bank width)

f32 = mybir.dt.float32
f32r = mybir.dt.float32r

_CACHE = {}


def _build():
    nc = bacc.Bacc()
    x = nc.declare_dram_parameter("x", [D, S], f32, isOutput=False)
    gT = nc.declare_dram_parameter("gT", [D, F], f32, isOutput=False)
    uT = nc.declare_dram_parameter("uT", [D, F], f32, isOutput=False)
    dT = nc.declare_dram_parameter("dT", [F, D], f32, isOutput=False)
    gB = nc.declare_dram_parameter("gB", [R, D], f32, isOutput=False)
    uB = nc.declare_dram_parameter("uB", [R, D], f32, isOutput=False)
    gAT = nc.declare_dram_parameter("gAT", [R, F], f32, isOutput=False)
    uAT = nc.declare_dram_parameter("uAT", [R, F], f32, isOutput=False)
    dBs = nc.declare_dram_parameter("dBs", [R, F], f32, isOutput=False)
    dAT = nc.declare_dram_parameter("dAT", [R, D], f32, isOutput=False)
    out = nc.declare_dram_parameter("out", [D, S], f32, isOutput=True)

    with TileContext(nc) as tc:
        with (
            tc.tile_pool(name="const", bufs=1) as const,
            tc.tile_pool(name="stage", bufs=2) as stage,
            tc.tile_pool(name="wstream", bufs=6) as wpool,
            tc.tile_pool(name="hbuf", bufs=1) as hpool,
            tc.tile_pool(name="obuf", bufs=3) as opool,
            tc.tile_pool(name="psacc", bufs=1, space="PSUM") as psacc,
            tc.tile_pool(name="pssc", bufs=2, space="PSUM") as pssc,
        ):
            # x resident in SBUF (f32r-rounded) as 4 tiles of [128, 4, 512]
            x_sb = []
            for xc in range(4):
                xst = stage.tile([P, 4, S], f32, name=f"xst{xc}", tag="xst")
                nc.sync.dma_start(
                    xst, x[xc * 4 * P:(xc + 1) * 4 * P, :].rearrange(
                        "(ko p) s -> p ko s", p=P)
                )
                xt = const.tile([P, 4, S], f32r, name=f"x{xc}", tag=f"x{xc}")
                nc.scalar.copy(xt, xst)
                x_sb.append(xt)

            def xs(kd):
                return x_sb[kd // 4][:, kd % 4]

            # low-rank scale factors, resident, f32r-rounded
            names = ["gB", "uB", "gAT", "uAT", "dBs", "dAT"]
            drams = [gB, uB, gAT, uAT, dBs, dAT]
            rounded = {}
            for nm, dram in zip(names, drams):
                st = stage.tile([R, dram.shape[1]], f32, name=f"{nm}st", tag=f"{nm}st")
                nc.sync.dma_start(st, dram[:])
                rt = const.tile([R, dram.shape[1]], f32r, name=f"{nm}r", tag=f"{nm}r")
                nc.vector.tensor_copy(out=rt, in_=st)
                rounded[nm] = rt
            gB_sb, uB_sb = rounded["gB"], rounded["uB"]
            gAT_sb, uAT_sb = rounded["gAT"], rounded["uAT"]
            dBs_sb, dAT_sb = rounded["dBs"], rounded["dAT"]

            # h = silu(gate) * up, [128, 8, 512] resident, f32r
            h_sb = hpool.tile([P, KF, S], f32r)

            silu = mybir.ActivationFunctionType.Silu

            def weight_pass(wdram, B_sb, AT_sb, nk, fg, rhs_fn, finish_fn):
                """One [128*nk, FG]-weight pass: stream+dequant the weight,
                accumulate 4 [128, S] outputs over the contraction dim.

                Scale matmuls are emitted one iteration ahead so the PE can
                keep running main matmuls while the DVE dequants."""
                acc = [psacc.tile([P, S], f32, name=f"acc{i}", tag=f"acc{i}")
                       for i in range(4)]
                sc_tiles = {}

                def emit_sc(k):
                    sc = pssc.tile([P, FG], f32, name="sc", tag="sc")
                    nc.tensor.matmul(
                        sc,
                        B_sb[:, k * P:(k + 1) * P],
                        AT_sb[:, fg * FG:(fg + 1) * FG],
                        start=True, stop=True,
                    )
                    sc_tiles[k] = sc

                emit_sc(0)
                for k in range(nk):
                    wt = wpool.tile([P, FG], f32, name="wt", tag="wt")
                    nc.sync.dma_start(
                        wt, wdram[k * P:(k + 1) * P, fg * FG:(fg + 1) * FG])
                    if k + 1 < nk:
                        emit_sc(k + 1)
                    wr = wpool.tile([P, FG], f32r, name="wr", tag="wr")
                    nc.vector.tensor_mul(out=wr, in0=wt, in1=sc_tiles.pop(k))
                    for fi in range(4):
                        nc.tensor.matmul(
                            acc[fi],
                            wr[:, fi * P:(fi + 1) * P],
                            rhs_fn(k),
                            start=(k == 0), stop=(k == nk - 1),
                        )
                finish_fn(acc)

            # ---- gate pass then up pass ----
            for is_up in (0, 1):
                wT = uT if is_up else gT
                B_sb = uB_sb if is_up else gB_sb
                AT_sb = uAT_sb if is_up else gAT_sb
                for fg in range(F // FG):

                    def finish(acc, fg=fg, is_up=is_up):
                        for fi in range(4):
                            f = fg * 4 + fi
                            if is_up:
                                nc.vector.tensor_mul(
                                    out=h_sb[:, f], in0=h_sb[:, f], in1=acc[fi])
                            else:
                                nc.scalar.activation(h_sb[:, f], acc[fi], silu)

                    weight_pass(wT, B_sb, AT_sb, KD, fg, xs, finish)

            # ---- down pass ----
            for mg in range(D // FG):

                def finish(acc, mg=mg):
                    for mi in range(4):
                        m = mg * 4 + mi
                        ot = opool.tile([P, S], f32, name="ot", tag="ot")
                        nc.scalar.copy(ot, acc[mi])
                        nc.sync.dma_start(out[m * P:(m + 1) * P, :], ot)

                weight_pass(dT, dBs_sb, dAT_sb, KF, mg,
                            lambda kf: h_sb[:, kf], finish)
    nc.finalize()
    return nc


def _prep_inputs(x, gate_snapped, gate_scale_A, gate_scale_B,
                 up_snapped, up_scale_A, up_scale_B,
                 down_snapped, down_scale_A, down_scale_B):
    asf = lambda a: np.ascontiguousarray(np.asarray(a, dtype=np.float32))
    x2 = asf(x).reshape(D, S)
    gT_full = asf(gate_snapped).T      # [D, FF] view
    uT_full = asf(up_snapped).T
    dT_full = asf(down_snapped).T      # [FF, D] view
    gB = asf(gate_scale_B)
    uB = asf(up_scale_B)
    dB = asf(down_scale_B)
    gA = asf(gate_scale_A)
    uA = asf(up_scale_A)
    dAT = np.ascontiguousarray(asf(down_scale_A).T)

    in_maps = []
    for c in range(NCORES):
        lo, hi = c * F, (c + 1) * F
        in_maps.append({
            "x": x2,
            "gT": np.ascontiguousarray(gT_full[:, lo:hi]),
            "uT": np.ascontiguousarray(uT_full[:, lo:hi]),
            "dT": np.ascontiguousarray(dT_full[lo:hi, :]),
            "gB": gB,
            "uB": uB,
            "gAT": np.ascontiguousarray(gA[lo:hi].T),
            "uAT": np.ascontiguousarray(uA[lo:hi].T),
            "dBs": np.ascontiguousarray(dB[:, lo:hi]),
            "dAT": dAT,
        })
    return in_maps


def run(trace=False, **inputs):
    if "nc" not in _CACHE:
        _CACHE["nc"] = _build()
    nc = _CACHE["nc"]
    in_maps = _prep_inputs(**inputs)
    res = run_bass_kernel_spmd(nc, in_maps, list(range(NCORES)), trace=trace)
    partial = np.zeros((D, S), dtype=np.float32)
    for c in range(NCORES):
        partial += res.results[c]["out"]
    return partial.reshape(1, D, 1, S), res


def kernel(**inputs):
    out, _ = run(trace=False, **inputs)
    return out


if __name__ == "__main__":
    rng = np.random.default_rng(0)
    ins = {
        "x": rng.standard_normal((1, D, 1, S)).astype(np.float32),
        "gate_snapped": (rng.standard_normal((FF, D)) * 0.02).astype(np.float32),
        "gate_scale_A": (rng.standard_normal((FF, R)) * 0.1).astype(np.float32),
        "gate_scale_B": (rng.standard_normal((R, D)) * 0.1).astype(np.float32),
        "up_snapped": (rng.standard_normal((FF, D)) * 0.02).astype(np.float32),
        "up_scale_A": (rng.standard_normal((FF, R)) * 0.1).astype(np.float32),
        "up_scale_B": (rng.standard_normal((R, D)) * 0.1).astype(np.float32),
        "down_snapped": (rng.standard_normal((D, FF)) * 0.02).astype(np.float32),
        "down_scale_A": (rng.standard_normal((D, R)) * 0.1).astype(np.float32),
        "down_scale_B": (rng.standard_normal((R, FF)) * 0.1).astype(np.float32),
    }
    out = kernel(**ins)
    print("kernel ran, out shape", out.shape, "mean abs", np.abs(out).mean())


# revision 5
# speedup vs baseline: 1.2646x; 1.2646x over previous
"""Trainium2 Bass kernel for FFNWithScales (SwiGLU MLP with low-rank dequant scales).

Reference computation (all fp32):
    gate_eff = gate_snapped * (gate_scale_A @ gate_scale_B)       # [8192, 2048]
    up_eff   = up_snapped   * (up_scale_A   @ up_scale_B)         # [8192, 2048]
    down_eff = down_snapped * (down_scale_A @ down_scale_B)       # [2048, 8192]
    h   = silu(gate_eff @ x) * (up_eff @ x)                       # [8192, 512]
    out = down_eff @ h                                            # [2048, 512]

Sharding (8 cores, tensor-parallel on d_ff): core c owns d_ff rows
[c*1024, (c+1)*1024) of gate/up (and the matching columns of down).
Each core computes a full-[2048, 512] partial of the down projection;
partials are summed on the host (the all-reduce step).

Device notes:
  - PE matmul computes psum[M,N] = lhsT[K,M].T @ rhs[K,N] with K on
    partitions, so every weight is fed with its contraction dim on
    partitions. The host pre-transposes the snapped weights (one numpy
    transpose each) because fp32 has no DMA-transpose path on TRN2.
  - The low-rank scale (A@B)^T tiles are produced on the PE (K=32
    matmuls) and multiplied into the snapped fp32 tiles on the DVE
    right before they are consumed as matmul weights.
  - Matmul operands are rounded to MM_DT at the last producing op
    (dequant multiply / silu / copies). bf16 streams ~3x faster than
    fp32r on the PE and is what makes this kernel DMA-bound; psum
    accumulation stays fp32 either way. Measured end-to-end error vs
    the fp32 reference: ~5e-3 of output absmax (bf16), ~3e-4 (f32r).
"""

import numpy as np

import concourse.bass as bass
from concourse import bacc
import concourse.mybir as mybir
from concourse.tile import TileContext
from concourse.bass_utils import run_bass_kernel_spmd

P = 128
D = 2048        # d_model
FF = 8192       # d_ff (global)
S = 512         # sequence
R = 32          # rank
NCORES = 8
F = FF // NCORES          # 1024 local d_ff rows
KD = D // P               # 16 d_model chunks
KF = F // P               # 8 local d_ff chunks
FG = 512                  # free-dim group (psum bank width)

f32 = mybir.dt.float32
MM_DT = mybir.dt.bfloat16  # matmul operand dtype (bfloat16 | float32r)

_CACHE = {}


def _build():
    nc = bacc.Bacc()
    x = nc.declare_dram_parameter("x", [D, S], f32, isOutput=False)
    gT = nc.declare_dram_parameter("gT", [D, F], f32, isOutput=False)
    uT = nc.declare_dram_parameter("uT", [D, F], f32, isOutput=False)
    dT = nc.declare_dram_parameter("dT", [F, D], f32, isOutput=False)
    gB = nc.declare_dram_parameter("gB", [R, D], f32, isOutput=False)
    uB = nc.declare_dram_parameter("uB", [R, D], f32, isOutput=False)
    gAT = nc.declare_dram_parameter("gAT", [R, F], f32, isOutput=False)
    uAT = nc.declare_dram_parameter("uAT", [R, F], f32, isOutput=False)
    dBs = nc.declare_dram_parameter("dBs", [R, F], f32, isOutput=False)
    dAT = nc.declare_dram_parameter("dAT", [R, D], f32, isOutput=False)
    out = nc.declare_dram_parameter("out", [D, S], f32, isOutput=True)

    with TileContext(nc) as tc:
        with (
            tc.tile_pool(name="const", bufs=1) as const,
            tc.tile_pool(name="stage", bufs=2) as stage,
            tc.tile_pool(name="wstream", bufs=8) as wpool,
            tc.tile_pool(name="hbuf", bufs=1) as hpool,
            tc.tile_pool(name="obuf", bufs=3) as opool,
            tc.tile_pool(name="psacc", bufs=1, space="PSUM") as psacc,
            tc.tile_pool(name="pssc", bufs=2, space="PSUM") as pssc,
        ):
            # x lives in SBUF (rounded to MM_DT) as 8 tiles of [128, 2, 512].
            # Loads are interleaved into the first weight pass (see below)
            # so the weight stream isn't stuck behind 4 MiB of x DMA.
            XC = 2                   # kd-chunks per x tile
            x_sb = [None] * (KD // XC)

            def load_x_chunk(q):
                xst = stage.tile([P, XC, S], f32, name=f"xst{q}", tag="xst")
                nc.sync.dma_start(
                    xst, x[q * XC * P:(q + 1) * XC * P, :].rearrange(
                        "(ko p) s -> p ko s", p=P))
                xt = const.tile([P, XC, S], MM_DT, name=f"x{q}", tag=f"x{q}")
                nc.scalar.copy(xt, xst)
                x_sb[q] = xt

            def xs(kd):
                return x_sb[kd // XC][:, kd % XC]

            load_x_chunk(0)
            load_x_chunk(1)

            # low-rank scale factors, resident, rounded
            names = ["gB", "uB", "gAT", "uAT", "dBs", "dAT"]
            drams = [gB, uB, gAT, uAT, dBs, dAT]
            rounded = {}
            for nm, dram in zip(names, drams):
                st = stage.tile([R, dram.shape[1]], f32, name=f"{nm}st", tag=f"{nm}st")
                nc.sync.dma_start(st, dram[:])
                rt = const.tile([R, dram.shape[1]], MM_DT, name=f"{nm}r", tag=f"{nm}r")
                nc.vector.tensor_copy(out=rt, in_=st)
                rounded[nm] = rt
            gB_sb, uB_sb = rounded["gB"], rounded["uB"]
            gAT_sb, uAT_sb = rounded["gAT"], rounded["uAT"]
            dBs_sb, dAT_sb = rounded["dBs"], rounded["dAT"]

            # h = silu(gate) * up, [128, 8, 512] resident
            h_sb = hpool.tile([P, KF, S], MM_DT)

            silu = mybir.ActivationFunctionType.Silu

            def weight_pass(wdram, B_sb, AT_sb, nk, fg, rhs_fn, finish_fn,
                            per_iter=None):
                """One [128*nk, FG]-weight pass: stream+dequant the weight,
                accumulate 4 [128, S] outputs over the contraction dim.

                Scale matmuls are emitted one iteration ahead so the PE can
                keep running main matmuls while the DVE dequants."""
                acc = [psacc.tile([P, S], f32, name=f"acc{i}", tag=f"acc{i}")
                       for i in range(4)]
                sc_tiles = {}

                def emit_sc(k):
                    sc = pssc.tile([P, FG], f32, name="sc", tag="sc")
                    nc.tensor.matmul(
                        sc,
                        B_sb[:, k * P:(k + 1) * P],
                        AT_sb[:, fg * FG:(fg + 1) * FG],
                        start=True, stop=True,
                    )
                    sc_tiles[k] = sc

                emit_sc(0)
                for k in range(nk):
                    wt = wpool.tile([P, FG], f32, name="wt", tag="wt")
                    nc.sync.dma_start(
                        wt, wdram[k * P:(k + 1) * P, fg * FG:(fg + 1) * FG])
                    if per_iter is not None:
                        per_iter(k)
                    if k + 1 < nk:
                        emit_sc(k + 1)
                    wr = wpool.tile([P, FG], MM_DT, name="wr", tag="wr")
                    nc.vector.tensor_mul(out=wr, in0=wt, in1=sc_tiles.pop(k))
                    for fi in range(4):
                        nc.tensor.matmul(
                            acc[fi],
                            wr[:, fi * P:(fi + 1) * P],
                            rhs_fn(k),
                            start=(k == 0), stop=(k == nk - 1),
                        )
                finish_fn(acc)

            def first_pass_iter(k):
                # pull the remaining x chunks in behind the weight stream:
                # chunk q first used at k=2q; emit its DMA at k=2q-3
                q = (k + 3) // 2
                if k % 2 == 1 and 2 <= q < KD // XC:
                    load_x_chunk(q)

            # ---- gate pass then up pass ----
            for is_up in (0, 1):
                wT = uT if is_up else gT
                B_sb = uB_sb if is_up else gB_sb
                AT_sb = uAT_sb if is_up else gAT_sb
                for fg in range(F // FG):

                    def finish(acc, fg=fg, is_up=is_up):
                        for fi in range(4):
                            f = fg * 4 + fi
                            if is_up:
                                nc.vector.tensor_mul(
                                    out=h_sb[:, f], in0=h_sb[:, f], in1=acc[fi])
                            else:
                                nc.scalar.activation(h_sb[:, f], acc[fi], silu)

                    weight_pass(
                        wT, B_sb, AT_sb, KD, fg, xs, finish,
                        per_iter=first_pass_iter if (is_up, fg) == (0, 0) else None)

            # ---- down pass ----
            for mg in range(D // FG):

                def finish(acc, mg=mg):
                    for mi in range(4):
                        m = mg * 4 + mi
                        ot = opool.tile([P, S], f32, name="ot", tag="ot")
                        nc.scalar.copy(ot, acc[mi])
                        nc.sync.dma_start(out[m * P:(m + 1) * P, :], ot)

                weight_pass(dT, dBs_sb, dAT_sb, KF, mg,
                            lambda kf: h_sb[:, kf], finish)
    nc.finalize()
    return nc


def _prep_inputs(x, gate_snapped, gate_scale_A, gate_scale_B,
                 up_snapped, up_scale_A, up_scale_B,
                 down_snapped, down_scale_A, down_scale_B):
    asf = lambda a: np.ascontiguousarray(np.asarray(a, dtype=np.float32))
    x2 = asf(x).reshape(D, S)
    gT_full = asf(gate_snapped).T      # [D, FF] view
    uT_full = asf(up_snapped).T
    dT_full = asf(down_snapped).T      # [FF, D] view
    gB = asf(gate_scale_B)
    uB = asf(up_scale_B)
    dB = asf(down_scale_B)
    gA = asf(gate_scale_A)
    uA = asf(up_scale_A)
    dAT = np.ascontiguousarray(asf(down_scale_A).T)

    in_maps = []
    for c in range(NCORES):
        lo, hi = c * F, (c + 1) * F
        in_maps.append({
            "x": x2,
            "gT": np.ascontiguousarray(gT_full[:, lo:hi]),
            "uT": np.ascontiguousarray(uT_full[:, lo:hi]),
            "dT": np.ascontiguousarray(dT_full[lo:hi, :]),
            "gB": gB,
            "uB": uB,
            "gAT": np.ascontiguousarray(gA[lo:hi].T),
            "uAT": np.ascontiguousarray(uA[lo:hi].T),
            "dBs": np.ascontiguousarray(dB[:, lo:hi]),
            "dAT": dAT,
        })
    return in_maps


def run(trace=False, **inputs):
    if "nc" not in _CACHE:
        _CACHE["nc"] = _build()
    nc = _CACHE["nc"]
    in_maps = _prep_inputs(**inputs)
    res = run_bass_kernel_spmd(nc, in_maps, list(range(NCORES)), trace=trace)
    partial = np.zeros((D, S), dtype=np.float32)
    for c in range(NCORES):
        partial += res.results[c]["out"]
    return partial.reshape(1, D, 1, S), res


def kernel(**inputs):
    out, _ = run(trace=False, **inputs)
    return out


if __name__ == "__main__":
    rng = np.random.default_rng(0)
    ins = {
        "x": rng.standard_normal((1, D, 1, S)).astype(np.float32),
        "gate_snapped": (rng.standard_normal((FF, D)) * 0.02).astype(np.float32),
        "gate_scale_A": (rng.standard_normal((FF, R)) * 0.1).astype(np.float32),
        "gate_scale_B": (rng.standard_normal((R, D)) * 0.1).astype(np.float32),
        "up_snapped": (rng.standard_normal((FF, D)) * 0.02).astype(np.float32),
        "up_scale_A": (rng.standard_normal((FF, R)) * 0.1).astype(np.float32),
        "up_scale_B": (rng.standard_normal((R, D)) * 0.1).astype(np.float32),
        "down_snapped": (rng.standard_normal((D, FF)) * 0.02).astype(np.float32),
        "down_scale_A": (rng.standard_normal((D, R)) * 0.1).astype(np.float32),
        "down_scale_B": (rng.standard_normal((R, FF)) * 0.1).astype(np.float32),
    }
    out = kernel(**ins)
    print("kernel ran, out shape", out.shape, "mean abs", np.abs(out).mean())


# revision 6
# speedup vs baseline: 1.3414x; 1.0607x over previous
"""Trainium2 Bass kernel for FFNWithScales (SwiGLU MLP with low-rank dequant scales).

Reference computation (all fp32):
    gate_eff = gate_snapped * (gate_scale_A @ gate_scale_B)       # [8192, 2048]
    up_eff   = up_snapped   * (up_scale_A   @ up_scale_B)         # [8192, 2048]
    down_eff = down_snapped * (down_scale_A @ down_scale_B)       # [2048, 8192]
    h   = silu(gate_eff @ x) * (up_eff @ x)                       # [8192, 512]
    out = down_eff @ h                                            # [2048, 512]

Sharding (8 cores, tensor-parallel on d_ff): core c owns d_ff rows
[c*1024, (c+1)*1024) of gate/up (and the matching columns of down).
Each core computes a full-[2048, 512] partial of the down projection;
partials are summed on the host (the all-reduce step).

Device notes:
  - PE matmul computes psum[M,N] = lhsT[K,M].T @ rhs[K,N] with K on
    partitions, so every weight is fed with its contraction dim on
    partitions. The host pre-transposes the snapped weights (one numpy
    transpose each) because fp32 has no DMA-transpose path on TRN2.
  - Weights stream through in [128, 2, 512] pairs: one 512 KiB DMA,
    one packed pair of rank-32 scale matmuls (row-tiled via
    tile_position so both run concurrently in the PE array), one DVE
    dequant multiply, eight [128,128]x[128,512] main matmuls.
  - Matmul operands are rounded to MM_DT at the last producing op
    (dequant multiply / silu / copies). bf16 streams ~3x faster than
    fp32r on the PE and is what makes this kernel DMA-bound; psum
    accumulation stays fp32 either way. Measured end-to-end error vs
    the fp32 reference: ~5e-3 of output absmax (bf16), ~3e-4 (f32r).
"""

import numpy as np

import concourse.bass as bass
from concourse import bacc
import concourse.mybir as mybir
from concourse.tile import TileContext
from concourse.bass_utils import run_bass_kernel_spmd

P = 128
D = 2048        # d_model
FF = 8192       # d_ff (global)
S = 512         # sequence
R = 32          # rank
NCORES = 8
F = FF // NCORES          # 1024 local d_ff rows
KD = D // P               # 16 d_model chunks
KF = F // P               # 8 local d_ff chunks
FG = 512                  # free-dim group (psum bank width)

f32 = mybir.dt.float32
MM_DT = mybir.dt.bfloat16  # matmul operand dtype (bfloat16 | float32r)

_CACHE = {}


def _build():
    nc = bacc.Bacc()
    x = nc.declare_dram_parameter("x", [D, S], f32, isOutput=False)
    gT = nc.declare_dram_parameter("gT", [D, F], f32, isOutput=False)
    uT = nc.declare_dram_parameter("uT", [D, F], f32, isOutput=False)
    dT = nc.declare_dram_parameter("dT", [F, D], f32, isOutput=False)
    gB = nc.declare_dram_parameter("gB", [R, D], f32, isOutput=False)
    uB = nc.declare_dram_parameter("uB", [R, D], f32, isOutput=False)
    gAT = nc.declare_dram_parameter("gAT", [R, F], f32, isOutput=False)
    uAT = nc.declare_dram_parameter("uAT", [R, F], f32, isOutput=False)
    dBs = nc.declare_dram_parameter("dBs", [R, F], f32, isOutput=False)
    dAT = nc.declare_dram_parameter("dAT", [R, D], f32, isOutput=False)
    out = nc.declare_dram_parameter("out", [D, S], f32, isOutput=True)

    with TileContext(nc) as tc:
        with (
            tc.tile_pool(name="const", bufs=1) as const,
            tc.tile_pool(name="stage", bufs=2) as stage,
            tc.tile_pool(name="wstream", bufs=6) as wpool,
            tc.tile_pool(name="hbuf", bufs=1) as hpool,
            tc.tile_pool(name="obuf", bufs=3) as opool,
            tc.tile_pool(name="psacc", bufs=1, space="PSUM") as psacc,
            tc.tile_pool(name="pssc", bufs=2, space="PSUM") as pssc,
        ):
            # x lives in SBUF (rounded to MM_DT) as 8 tiles of [128, 2, 512].
            # Loads are interleaved into the first weight pass (see below)
            # so the weight stream isn't stuck behind 4 MiB of x DMA.
            XC = 2                   # kd-chunks per x tile
            x_sb = [None] * (KD // XC)

            def load_x_chunk(q):
                xst = stage.tile([P, XC, S], f32, name=f"xst{q}", tag="xst")
                nc.sync.dma_start(
                    xst, x[q * XC * P:(q + 1) * XC * P, :].rearrange(
                        "(ko p) s -> p ko s", p=P))
                xt = const.tile([P, XC, S], MM_DT, name=f"x{q}", tag=f"x{q}")
                nc.scalar.copy(xt, xst)
                x_sb[q] = xt

            def xs(kd):
                return x_sb[kd // XC][:, kd % XC]

            load_x_chunk(0)
            load_x_chunk(1)

            # Low-rank factors, laid out for row-tiled packed scale matmuls:
            #   B2 [64, nk/2, 128]: strip i (partitions 32i..32i+31) holds the
            #     B columns for kd-chunks 2*kp+i -> lhsT of the packed matmul.
            #   AT2 [64, width]: A^T replicated on both strips -> rhs.
            rounded = {}

            def load_B2(nm, dram, nk):
                st = stage.tile([2 * R, nk // 2, P], f32, name=f"{nm}st",
                                tag=f"{nm}st")
                src = dram[:].rearrange("r (kp rest) -> r kp rest", kp=nk // 2)
                for i in range(2):
                    nc.sync.dma_start(
                        st[i * R:(i + 1) * R], src[:, :, i * P:(i + 1) * P])
                rt = const.tile([2 * R, nk // 2, P], MM_DT, name=f"{nm}r",
                                tag=f"{nm}r")
                nc.vector.tensor_copy(out=rt, in_=st)
                rounded[nm] = rt

            def load_AT2(nm, dram):
                w = dram.shape[1]
                st = stage.tile([2 * R, w], f32, name=f"{nm}st", tag=f"{nm}st")
                for i in range(2):
                    nc.sync.dma_start(st[i * R:(i + 1) * R], dram[:])
                rt = const.tile([2 * R, w], MM_DT, name=f"{nm}r", tag=f"{nm}r")
                nc.vector.tensor_copy(out=rt, in_=st)
                rounded[nm] = rt

            load_B2("gB", gB, KD)
            load_AT2("gAT", gAT)

            # h = silu(gate) * up, [128, 8, 512] resident
            h_sb = hpool.tile([P, KF, S], MM_DT)

            silu = mybir.ActivationFunctionType.Silu

            def weight_pass(wdram, B2, AT2, nk, fg, rhs_fn, finish_fn,
                            per_iter=None):
                """One [128*nk, FG]-weight pass in pairs of kd-chunks:
                512 KiB weight DMA + packed scale matmuls + one dequant +
                8 accumulating matmuls per pair. Scale matmuls run one pair
                ahead so the PE never waits on the DVE dequant."""
                npairs = nk // 2
                acc = [psacc.tile([P, S], f32, name=f"acc{i}", tag=f"acc{i}")
                       for i in range(4)]
                sc_tiles = {}

                def emit_sc(kp):
                    sc2 = pssc.tile([P, 2, FG], f32, name="sc", tag="sc")
                    for i in range(2):
                        nc.tensor.matmul(
                            sc2[:, i],
                            B2()[i * R:(i + 1) * R, kp],
                            AT2()[i * R:(i + 1) * R, fg * FG:(fg + 1) * FG],
                            start=True, stop=True,
                            tile_position=(R * i, 0),
                        )
                    sc_tiles[kp] = sc2

                emit_sc(0)
                for kp in range(npairs):
                    wt2 = wpool.tile([P, 2, FG], f32, name="wt", tag="wt")
                    nc.sync.dma_start(
                        wt2,
                        wdram[kp * 2 * P:(kp + 1) * 2 * P,
                              fg * FG:(fg + 1) * FG].rearrange(
                                  "(ko p) f -> p ko f", p=P))
                    if per_iter is not None:
                        per_iter(kp)
                    if kp + 1 < npairs:
                        emit_sc(kp + 1)
                    wr2 = wpool.tile([P, 2, FG], MM_DT, name="wr", tag="wr")
                    nc.vector.tensor_mul(out=wr2, in0=wt2, in1=sc_tiles.pop(kp))
                    for j in range(2):
                        for fi in range(4):
                            nc.tensor.matmul(
                                acc[fi],
                                wr2[:, j, fi * P:(fi + 1) * P],
                                rhs_fn(2 * kp + j),
                                start=(kp == 0 and j == 0),
                                stop=(kp == npairs - 1 and j == 1),
                            )
                finish_fn(acc)

            def first_pass_iter(kp):
                # behind the first pairs of gate weights: pull in the rest of
                # x (chunk q first used at pair kp=q; emitted 2 pairs early)
                # and the up/down scale factors.
                q = kp + 2
                if 2 <= q < KD // XC:
                    load_x_chunk(q)
                if kp == 2:
                    load_B2("uB", uB, KD)
                    load_AT2("uAT", uAT)
                if kp == 4:
                    load_B2("dBs", dBs, KF)
                    load_AT2("dAT", dAT)

            # ---- gate pass then up pass ----
            for is_up in (0, 1):
                wT = uT if is_up else gT
                Bn = "uB" if is_up else "gB"
                An = "uAT" if is_up else "gAT"
                for fg in range(F // FG):

                    def finish(acc, fg=fg, is_up=is_up):
                        for fi in range(4):
                            f = fg * 4 + fi
                            if is_up:
                                nc.vector.tensor_mul(
                                    out=h_sb[:, f], in0=h_sb[:, f], in1=acc[fi])
                            else:
                                nc.scalar.activation(h_sb[:, f], acc[fi], silu)

                    weight_pass(
                        wT, lambda Bn=Bn: rounded[Bn], lambda An=An: rounded[An],
                        KD, fg, xs, finish,
                        per_iter=first_pass_iter if (is_up, fg) == (0, 0) else None)

            # ---- down pass ----
            for mg in range(D // FG):

                def finish(acc, mg=mg):
                    for mi in range(4):
                        m = mg * 4 + mi
                        ot = opool.tile([P, S], f32, name="ot", tag="ot")
                        nc.scalar.copy(ot, acc[mi])
                        nc.sync.dma_start(out[m * P:(m + 1) * P, :], ot)

                weight_pass(dT, lambda: rounded["dBs"], lambda: rounded["dAT"],
                            KF, mg, lambda kf: h_sb[:, kf], finish)
    nc.finalize()
    return nc


def _prep_inputs(x, gate_snapped, gate_scale_A, gate_scale_B,
                 up_snapped, up_scale_A, up_scale_B,
                 down_snapped, down_scale_A, down_scale_B):
    asf = lambda a: np.ascontiguousarray(np.asarray(a, dtype=np.float32))
    x2 = asf(x).reshape(D, S)
    gT_full = asf(gate_snapped).T      # [D, FF] view
    uT_full = asf(up_snapped).T
    dT_full = asf(down_snapped).T      # [FF, D] view
    gB = asf(gate_scale_B)
    uB = asf(up_scale_B)
    dB = asf(down_scale_B)
    gA = asf(gate_scale_A)
    uA = asf(up_scale_A)
    dAT = np.ascontiguousarray(asf(down_scale_A).T)

    in_maps = []
    for c in range(NCORES):
        lo, hi = c * F, (c + 1) * F
        in_maps.append({
            "x": x2,
            "gT": np.ascontiguousarray(gT_full[:, lo:hi]),
            "uT": np.ascontiguousarray(uT_full[:, lo:hi]),
            "dT": np.ascontiguousarray(dT_full[lo:hi, :]),
            "gB": gB,
            "uB": uB,
            "gAT": np.ascontiguousarray(gA[lo:hi].T),
            "uAT": np.ascontiguousarray(uA[lo:hi].T),
            "dBs": np.ascontiguousarray(dB[:, lo:hi]),
            "dAT": dAT,
        })
    return in_maps


def run(trace=False, **inputs):
    if "nc" not in _CACHE:
        _CACHE["nc"] = _build()
    nc = _CACHE["nc"]
    in_maps = _prep_inputs(**inputs)
    res = run_bass_kernel_spmd(nc, in_maps, list(range(NCORES)), trace=trace)
    partial = np.zeros((D, S), dtype=np.float32)
    for c in range(NCORES):
        partial += res.results[c]["out"]
    return partial.reshape(1, D, 1, S), res


def kernel(**inputs):
    out, _ = run(trace=False, **inputs)
    return out


if __name__ == "__main__":
    rng = np.random.default_rng(0)
    ins = {
        "x": rng.standard_normal((1, D, 1, S)).astype(np.float32),
        "gate_snapped": (rng.standard_normal((FF, D)) * 0.02).astype(np.float32),
        "gate_scale_A": (rng.standard_normal((FF, R)) * 0.1).astype(np.float32),
        "gate_scale_B": (rng.standard_normal((R, D)) * 0.1).astype(np.float32),
        "up_snapped": (rng.standard_normal((FF, D)) * 0.02).astype(np.float32),
        "up_scale_A": (rng.standard_normal((FF, R)) * 0.1).astype(np.float32),
        "up_scale_B": (rng.standard_normal((R, D)) * 0.1).astype(np.float32),
        "down_snapped": (rng.standard_normal((D, FF)) * 0.02).astype(np.float32),
        "down_scale_A": (rng.standard_normal((D, R)) * 0.1).astype(np.float32),
        "down_scale_B": (rng.standard_normal((R, FF)) * 0.1).astype(np.float32),
    }
    out = kernel(**ins)
    print("kernel ran, out shape", out.shape, "mean abs", np.abs(out).mean())
